# revision 6
# baseline (speedup 1.0000x reference)
# Multi-head self-attention (B=2, T=2048, C=2048, H=16) on 8 trn2 NeuronCores.
# Sharding: core = (batch b, head-group g) with 4 heads per core.
# Per-core program (Tile framework, bf16 matmuls with fp32 PSUM accumulation):
#   qk^T = W_qk^T @ x^T   (lhsT = W chunks, rhs = x^T)      -> [D, T] per head
#   v    = x @ W_v        (lhsT = x^T chunks, rhs = W_v)    -> [T, D] natural
#   RoPE on q^T/k^T via half-swap DMA + elementwise mul/add
#   S^T tile = k_rope^T.T @ q_rope^T ; E^T = exp(scale*S^T) (causal-masked)
#   out^T = v.T @ E^T ; rowsums via ones-matmul ; normalize by bcast(1/sums)
#   y_partial = out_heads^T.T @ W_p rows  -> [T, C], host sums 4 partials per batch.
import os
import sys

import numpy as np
import ml_dtypes

for _p in ("/opt/trn_rl_repo",):
    if _p not in sys.path:
        sys.path.append(_p)

import concourse.bass as bass
import concourse.mybir as mybir
import concourse.tile as tile
from concourse import bacc
from concourse.bass_utils import run_bass_kernel_spmd

P = 128
T = 2048
C = 2048
D = 128
NH = 4            # heads per core
KO = C // P       # 16 contraction chunks
TQ = 512          # q-tile width
NQ = T // TQ      # 4
NT = T // P       # 16 t-subtiles
SCALE = float(np.float32(1.0) / np.sqrt(np.float32(D)))

F32 = mybir.dt.float32
BF16 = mybir.dt.bfloat16
AF = mybir.ActivationFunctionType

TRACE = False
_CACHED_NC = None


def _mask_np():
    # masks[p, j, q] for diagonal k-chunks: keep iff 128*j + p <= q (q in 0..511)
    p = np.arange(P)[:, None, None]
    j = np.arange(4)[None, :, None]
    q = np.arange(TQ)[None, None, :]
    return (128 * j + p <= q).astype(ml_dtypes.bfloat16)


def build_nc():
    nc = bacc.Bacc("TRN2", target_bir_lowering=False, debug=False,
                   enable_asserts=False)

    xT_d = nc.dram_tensor("xT", [C, T], F32, kind="ExternalInput")
    wqk_d = nc.dram_tensor("wqk", [C, 2 * NH * D], F32, kind="ExternalInput")
    wv_d = nc.dram_tensor("wv", [C, NH * D], F32, kind="ExternalInput")
    wp_d = nc.dram_tensor("wp", [NH * D, C], F32, kind="ExternalInput")
    cos_d = nc.dram_tensor("cosT", [D, T], F32, kind="ExternalInput")
    sin_d = nc.dram_tensor("sinT", [D, T], F32, kind="ExternalInput")
    y_d = nc.dram_tensor("y", [T, C], F32, kind="ExternalOutput")

    masks_d = nc.inline_tensor(_mask_np(), name="masks")

    xT = xT_d.ap().rearrange("(ko p) t -> p ko t", p=P)          # [128,16,2048]
    wqk = wqk_d.ap().rearrange("(ko p) m -> p ko m", p=P)        # [128,16,1024]
    wv = wv_d.ap().rearrange("(ko p) m -> p ko m", p=P)          # [128,16,512]
    wp = wp_d.ap().rearrange("(ho p) c -> p ho c", p=P)          # [128,4,2048]
    y = y_d.ap()

    with tile.TileContext(nc) as tc:
        with (
            tc.tile_pool(name="glob", bufs=1) as glob,
            tc.tile_pool(name="rawp", bufs=1) as rawp,
            tc.tile_pool(name="psB", bufs=2, space="PSUM") as psB,
            tc.tile_pool(name="psS", bufs=2, space="PSUM") as psS,
            tc.tile_pool(name="psO", bufs=2, space="PSUM") as psO,
            tc.tile_pool(name="psR", bufs=2, space="PSUM") as psR,
        ):
            # ---- constants ----
            cos_b = glob.tile([P, T], BF16, tag="cos_b")
            sin_b = glob.tile([P, T], BF16, tag="sin_b")
            masks_sb = glob.tile([P, 4, TQ], BF16, tag="masks")
            ones_sb = glob.tile([P, 1], BF16, tag="ones")
            nc.sync.dma_start(masks_sb[:], masks_d.ap())
            nc.vector.memset(ones_sb[:], 1.0)

            # persistent per-head tensors
            v_b = [glob.tile([P, NT, 130], BF16, tag=f"v_b{h}", name=f"v_b{h}")
                   for h in range(NH)]
            outT = [glob.tile([P, T], BF16, tag=f"outT{h}", name=f"outT{h}")
                    for h in range(NH)]

            # =============== Phase B: qkv matmuls ===============
            with tc.tile_pool(name="loadB", bufs=1) as lB, \
                 tc.tile_pool(name="stage", bufs=2) as stg, \
                 tc.tile_pool(name="xbp", bufs=18) as xbp, \
                 tc.tile_pool(name="shufp", bufs=2) as shufp:

                # cos/sin cast via 512-wide chunks through the x staging tag
                for nb in range(NQ):
                    sl = slice(nb * TQ, (nb + 1) * TQ)
                    cs = stg.tile([P, TQ], F32, tag="xstage", bufs=4, name="cs")
                    nc.sync.dma_start(cs[:], cos_d.ap()[:, sl])
                    nc.scalar.activation(cos_b[:, sl], cs[:], AF.Copy)
                    ss = stg.tile([P, TQ], F32, tag="xstage", bufs=4, name="ss")
                    nc.sync.dma_start(ss[:], sin_d.ap()[:, sl])
                    nc.scalar.activation(sin_b[:, sl], ss[:], AF.Copy)

                # weights -> bf16 resident
                wqk_b = []
                for m in range(8):
                    wb = lB.tile([P, KO, P], BF16, tag=f"wqk_b{m}", name=f"wqk_b{m}")
                    for half in range(2):
                        ksl = slice(half * (KO // 2), (half + 1) * (KO // 2))
                        wst = stg.tile([P, KO // 2, P], F32, tag="wqkstage",
                                       name="wst")
                        nc.sync.dma_start(wst[:], wqk[:, ksl, m * P:(m + 1) * P])
                        nc.scalar.activation(wb[:, ksl, :], wst[:], AF.Copy)
                    wqk_b.append(wb)
                wv_b = lB.tile([P, KO, NH * D], BF16, tag="wv_b")
                for ko in range(KO):
                    wvs = stg.tile([P, NH * D], F32, tag="wvstage", bufs=3,
                                   name="wvs")
                    nc.sync.dma_start(wvs[:], wv[:, ko, :])
                    nc.scalar.activation(wv_b[:, ko, :], wvs[:], AF.Copy)

                raw = [rawp.tile([P, T], BF16, tag=f"raw{m}", name=f"raw{m}")
                       for m in range(8)]

                for nblk in range(NQ):
                    tsl = slice(nblk * TQ, (nblk + 1) * TQ)
                    xb = []
                    for ko in range(KO):
                        xs = stg.tile([P, TQ], F32, tag="xstage", bufs=4,
                                      name="xs")
                        nc.sync.dma_start(xs[:], xT[:, ko, tsl])
                        xt = xbp.tile([P, TQ], BF16, tag="xb", name="xb")
                        nc.vector.tensor_copy(xt[:], xs[:])
                        xb.append(xt)
                    # qk matmuls: out [m*128 dims, 512 t]
                    for m in range(8):
                        ps = psB.tile([P, TQ], F32, tag="psBig")
                        for ko in range(KO):
                            nc.tensor.matmul(ps[:], lhsT=wqk_b[m][:, ko, :],
                                             rhs=xb[ko][:],
                                             start=(ko == 0), stop=(ko == KO - 1))
                        nc.scalar.activation(raw[m][:, tsl], ps[:], AF.Copy)
                    # v matmuls: natural [t 128, 512 = 4 heads * D]
                    for t4 in range(4):
                        tg = nblk * 4 + t4
                        ps = psB.tile([P, TQ], F32, tag="psBig")
                        for ko in range(KO):
                            nc.tensor.matmul(
                                ps[:], lhsT=xb[ko][:, t4 * P:(t4 + 1) * P],
                                rhs=wv_b[:, ko, :],
                                start=(ko == 0), stop=(ko == KO - 1))
                        for h in range(NH):
                            nc.scalar.activation(
                                v_b[h][:, tg, 0:P],
                                ps[:, h * P:(h + 1) * P], AF.Copy)

                # =============== Phase C: RoPE (in place on raw) ===============
                roped = {}
                for h in range(NH):
                    for m in (h, 4 + h):
                        r = raw[m]
                        sh = shufp.tile([P, T], BF16, tag="shuf", name="sh")
                        nc.sync.dma_start(sh[0:64, :], r[64:128, :])
                        nc.sync.dma_start(sh[64:128, :], r[0:64, :])
                        nc.vector.tensor_mul(sh[:], sh[:], sin_b[:])
                        nc.vector.tensor_mul(r[:], r[:], cos_b[:])
                        nc.vector.tensor_add(r[:], r[:], sh[:])
                        roped[m] = r

            # =============== Phase D: attention ===============
            with tc.tile_pool(name="attnp", bufs=1) as ap_, \
                 tc.tile_pool(name="etp", bufs=3) as etp, \
                 tc.tile_pool(name="nrm", bufs=2) as nrm, \
                 tc.tile_pool(name="ystg", bufs=3) as ystg, \
                 tc.tile_pool(name="wpstage", bufs=2) as wps:

                # load wp during attention (1024-wide chunks)
                wp_b = ap_.tile([P, NH, C], BF16, tag="wp_b")
                for ho in range(NH):
                    for half in range(2):
                        csl2 = slice(half * (C // 2), (half + 1) * (C // 2))
                        ws = wps.tile([P, C // 2], F32, tag="wpstage", name="ws")
                        nc.sync.dma_start(ws[:], wp[:, ho, csl2])
                        nc.scalar.activation(wp_b[:, ho, csl2], ws[:], AF.Copy)

                for h in range(NH):
                    qr = roped[h]
                    kr = roped[4 + h]
                    for qo in range(NQ):
                        qsl = slice(qo * TQ, (qo + 1) * TQ)
                        nk = 4 * (qo + 1)
                        ps_o = psO.tile([P, TQ], F32, tag="psout")
                        ps_r = psR.tile([1, TQ], F32, tag="psrow")
                        for j in range(nk):
                            ps_s = psS.tile([P, TQ], F32, tag="psscore")
                            nc.tensor.matmul(ps_s[:],
                                             lhsT=kr[:, j * P:(j + 1) * P],
                                             rhs=qr[:, qsl],
                                             start=True, stop=True)
                            et = etp.tile([P, TQ], BF16, tag="et")
                            nc.scalar.activation(et[:], ps_s[:], AF.Exp,
                                                 scale=SCALE)
                            if j >= nk - 4:
                                nc.vector.tensor_mul(
                                    et[:], et[:], masks_sb[:, j - (nk - 4), :])
                            nc.tensor.matmul(ps_o[:], lhsT=v_b[h][:, j, 0:P],
                                             rhs=et[:],
                                             start=(j == 0), stop=(j == nk - 1))
                            nc.tensor.matmul(ps_r[:], lhsT=ones_sb[:, 0:1],
                                             rhs=et[:],
                                             start=(j == 0), stop=(j == nk - 1))
                        sums = nrm.tile([1, TQ], F32, tag="sums")
                        nc.scalar.activation(sums[:], ps_r[:], AF.Copy)
                        recip = nrm.tile([1, TQ], F32, tag="recip")
                        nc.vector.reciprocal_approx_fast(recip[:], sums[:])
                        bcast = nrm.tile([P, TQ], F32, tag="bcast")
                        nc.gpsimd.partition_broadcast(bcast[:], recip[:])
                        nc.vector.tensor_mul(outT[h][:, qsl], ps_o[:], bcast[:])

                # =============== Phase F: projection ===============
                for t in range(NT):
                    for cn in range(NQ):
                        ps = psB.tile([P, TQ], F32, tag="psBig")
                        for h in range(NH):
                            nc.tensor.matmul(
                                ps[:], lhsT=outT[h][:, t * P:(t + 1) * P],
                                rhs=wp_b[:, h, cn * TQ:(cn + 1) * TQ],
                                start=(h == 0), stop=(h == NH - 1))
                        ys = ystg.tile([P, TQ], F32, tag="ystage")
                        nc.scalar.activation(ys[:], ps[:], AF.Copy)
                        nc.sync.dma_start(
                            y[t * P:(t + 1) * P, cn * TQ:(cn + 1) * TQ], ys[:])

    nc.compile()
    return nc


def _get_nc():
    global _CACHED_NC
    if _CACHED_NC is None:
        _CACHED_NC = build_nc()
    return _CACHED_NC


LAST_RESULTS = None


def kernel(x, cos, sin, W_attn, W_proj):
    global LAST_RESULTS
    x = np.asarray(x, np.float32)
    cos = np.asarray(cos, np.float32)
    sin = np.asarray(sin, np.float32)
    W_attn = np.asarray(W_attn, np.float32)
    W_proj = np.asarray(W_proj, np.float32)
    B = x.shape[0]

    cosT = np.ascontiguousarray(cos.T)                     # [D, T]
    sinT = np.ascontiguousarray(sin.T).copy()
    sinT[: D // 2] *= -1.0                                 # sign-folded rotate_half

    xTs = [np.ascontiguousarray(x[b].T) for b in range(B)]
    in_maps = []
    for b in range(B):
        for g in range(4):
            csl = slice(g * 512, (g + 1) * 512)
            wqk = np.ascontiguousarray(
                np.concatenate([W_attn[:, csl], W_attn[:, C:][:, csl]], axis=1))
            wv = np.ascontiguousarray(W_attn[:, 2 * C:][:, csl])
            wpg = np.ascontiguousarray(W_proj[g * 512:(g + 1) * 512, :])
            in_maps.append({"xT": xTs[b], "wqk": wqk, "wv": wv, "wp": wpg,
                            "cosT": cosT, "sinT": sinT})

    nc = _get_nc()
    res = run_bass_kernel_spmd(nc, in_maps, core_ids=list(range(8)),
                               trace=TRACE)
    LAST_RESULTS = res

    out = np.zeros((B, T, C), np.float32)
    for b in range(B):
        acc = res.results[b * 4 + 0]["y"].astype(np.float32)
        for g in range(1, 4):
            acc = acc + res.results[b * 4 + g]["y"]
        out[b] = acc
    return out


# revision 7
# speedup vs baseline: 1.0432x; 1.0432x over previous
# Multi-head self-attention (B=2, T=2048, C=2048, H=16) on 8 trn2 NeuronCores.
# Sharding: core = (batch b, head-group g) with 4 heads per core.
# Per-core program (Tile framework, bf16 matmuls with fp32 PSUM accumulation):
#   qk^T = W_qk^T @ x^T   (lhsT = W chunks, rhs = x^T)      -> [D, T] per head
#   v    = x @ W_v        (lhsT = x^T chunks, rhs = W_v)    -> [T, D] natural
#   RoPE on q^T/k^T via half-swap DMA + elementwise mul/add (in place)
#   S^T tile = k_rope^T.T @ q_rope^T ; E^T = exp(scale*S^T) (causal)
#   out^T = v.T @ E^T ; rowsums via ones-matmul ; normalize by bcast(1/sums)
#   y_partial = out_heads^T.T @ W_p rows  -> [T, C], host sums 4 partials.
import sys

import numpy as np
import ml_dtypes

for _p in ("/opt/trn_rl_repo",):
    if _p not in sys.path:
        sys.path.append(_p)

import concourse.bass as bass
import concourse.mybir as mybir
import concourse.tile as tile
from concourse import bacc
from concourse.bass_utils import run_bass_kernel_spmd

P = 128
T = 2048
C = 2048
D = 128
NH = 4            # heads per core
KO = C // P       # 16 contraction chunks
TQ = 512          # q-tile width
NQ = T // TQ      # 4
NT = T // P       # 16 t-subtiles
SCALE = float(np.float32(1.0) / np.sqrt(np.float32(D)))

F32 = mybir.dt.float32
BF16 = mybir.dt.bfloat16
AF = mybir.ActivationFunctionType

TRACE = False
_CACHED_NC = None


def _tri_mask_np():
    # keep iff p <= q  (k-row p of the 128-wide diagonal sub-block vs local q)
    p = np.arange(P)[:, None]
    q = np.arange(P)[None, :]
    return (p <= q).astype(ml_dtypes.bfloat16)


def build_nc():
    nc = bacc.Bacc("TRN2", target_bir_lowering=False, debug=False,
                   enable_asserts=False)

    xT_d = nc.dram_tensor("xT", [C, T], F32, kind="ExternalInput")
    wqk_d = nc.dram_tensor("wqk", [C, 2 * NH * D], F32, kind="ExternalInput")
    wv_d = nc.dram_tensor("wv", [C, NH * D], F32, kind="ExternalInput")
    wp_d = nc.dram_tensor("wp", [NH * D, C], F32, kind="ExternalInput")
    cos_d = nc.dram_tensor("cosT", [D, T], F32, kind="ExternalInput")
    sin_d = nc.dram_tensor("sinT", [D, T], F32, kind="ExternalInput")
    y_d = nc.dram_tensor("y", [T, C], F32, kind="ExternalOutput")

    mask_d = nc.inline_tensor(_tri_mask_np(), name="trimask")

    xT = xT_d.ap().rearrange("(ko p) t -> p ko t", p=P)          # [128,16,2048]
    wqk = wqk_d.ap().rearrange("(ko p) m -> p ko m", p=P)        # [128,16,1024]
    wv = wv_d.ap().rearrange("(ko p) m -> p ko m", p=P)          # [128,16,512]
    wp = wp_d.ap().rearrange("(ho p) c -> p ho c", p=P)          # [128,4,2048]
    y = y_d.ap()

    with tile.TileContext(nc) as tc:
        with (
            tc.tile_pool(name="glob", bufs=1) as glob,
            tc.tile_pool(name="rawp", bufs=1) as rawp,
        ):
            ones_sb = glob.tile([P, 1], BF16, tag="ones")
            nc.vector.memset(ones_sb[:], 1.0)
            v_b = [glob.tile([P, NT, P], BF16, tag=f"v_b{h}", name=f"v_b{h}")
                   for h in range(NH)]
            raw = [rawp.tile([P, T], BF16, tag=f"raw{m}", name=f"raw{m}")
                   for m in range(8)]

            # =============== Phase B: qkv matmuls + RoPE ===============
            with tc.tile_pool(name="loadB", bufs=1) as lB, \
                 tc.tile_pool(name="stage", bufs=4) as stg, \
                 tc.tile_pool(name="shufp", bufs=1) as shufp, \
                 tc.tile_pool(name="psB", bufs=6, space="PSUM") as psB:

                # x^T -> bf16, fully resident, ko-major so matmuls start early
                xb = lB.tile([P, KO, T], BF16, tag="xb")
                for ko in range(KO):
                    for half in range(2):
                        sl = slice(half * (T // 2), (half + 1) * (T // 2))
                        xs = stg.tile([P, T // 2], F32, tag="stg4k", name="xs")
                        nc.sync.dma_start(xs[:], xT[:, ko, sl])
                        nc.vector.tensor_copy(xb[:, ko, sl], xs[:])

                # qk weights -> bf16 resident
                wqk_b = []
                for m in range(8):
                    wb = lB.tile([P, KO, P], BF16, tag=f"wqk_b{m}",
                                 name=f"wqk_b{m}")
                    for half in range(2):
                        ksl = slice(half * (KO // 2), (half + 1) * (KO // 2))
                        wst = stg.tile([P, KO // 2, P], F32, tag="stg4k",
                                       name="wst")
                        nc.sync.dma_start(wst[:], wqk[:, ksl, m * P:(m + 1) * P])
                        nc.scalar.activation(wb[:, ksl, :], wst[:], AF.Copy)
                    wqk_b.append(wb)

                # cos/sin -> bf16 (needed by rope, which overlaps qk matmuls)
                cos_b = lB.tile([P, T], BF16, tag="cos_b")
                sin_b = lB.tile([P, T], BF16, tag="sin_b")
                for half in range(2):
                    sl = slice(half * (T // 2), (half + 1) * (T // 2))
                    cs = stg.tile([P, T // 2], F32, tag="stg4k", name="cs")
                    nc.sync.dma_start(cs[:], cos_d.ap()[:, sl])
                    nc.scalar.activation(cos_b[:, sl], cs[:], AF.Copy)
                    ss = stg.tile([P, T // 2], F32, tag="stg4k", name="ss")
                    nc.sync.dma_start(ss[:], sin_d.ap()[:, sl])
                    nc.scalar.activation(sin_b[:, sl], ss[:], AF.Copy)

                # qk matmuls, LDWEIGHTS amortized over the 4 T-tiles;
                # rope each raw[m] as soon as its evacuations are done.
                for m in range(8):
                    pss = [psB.tile([P, TQ], F32, tag="psBig", name="psqk")
                           for _ in range(NQ)]
                    for ko in range(KO):
                        for n in range(NQ):
                            nc.tensor.matmul(
                                pss[n][:], lhsT=wqk_b[m][:, ko, :],
                                rhs=xb[:, ko, n * TQ:(n + 1) * TQ],
                                start=(ko == 0), stop=(ko == KO - 1))
                    for n in range(NQ):
                        nc.scalar.activation(
                            raw[m][:, n * TQ:(n + 1) * TQ], pss[n][:], AF.Copy)
                    # rope in place
                    r = raw[m]
                    sh = shufp.tile([P, T], BF16, tag="shuf", name="sh")
                    nc.sync.dma_start(sh[0:64, :], r[64:128, :])
                    nc.sync.dma_start(sh[64:128, :], r[0:64, :])
                    nc.vector.tensor_mul(sh[:], sh[:], sin_b[:])
                    nc.vector.tensor_mul(r[:], r[:], cos_b[:])
                    nc.vector.tensor_add(r[:], r[:], sh[:])

                # v weights + v matmuls (natural layout)
                wv_b = lB.tile([P, KO, NH * D], BF16, tag="wv_b")
                for ko in range(KO):
                    wvs = stg.tile([P, NH * D], F32, tag="stg4k", name="wvs")
                    nc.sync.dma_start(wvs[:], wv[:, ko, :])
                    nc.scalar.activation(wv_b[:, ko, :], wvs[:], AF.Copy)
                for t in range(NT):
                    psv = psB.tile([P, TQ], F32, tag="psBig", name="psv")
                    for ko in range(KO):
                        nc.tensor.matmul(
                            psv[:], lhsT=xb[:, ko, t * P:(t + 1) * P],
                            rhs=wv_b[:, ko, :],
                            start=(ko == 0), stop=(ko == KO - 1))
                    for h in range(NH):
                        nc.scalar.activation(
                            v_b[h][:, t, :], psv[:, h * P:(h + 1) * P], AF.Copy)

            # =============== Phase D: attention ===============
            with tc.tile_pool(name="attnp", bufs=1) as ap_, \
                 tc.tile_pool(name="etp", bufs=3) as etp, \
                 tc.tile_pool(name="nrm", bufs=2) as nrm, \
                 tc.tile_pool(name="wps", bufs=2) as wps:

                mask_sb = ap_.tile([P, P], BF16, tag="trimask")
                nc.sync.dma_start(mask_sb[:], mask_d.ap())
                outT = [ap_.tile([P, T], BF16, tag=f"outT{h}", name=f"outT{h}")
                        for h in range(NH)]
                # load wp during attention (cast on vector - ACT is busy)
                wp_b = ap_.tile([P, NH, C], BF16, tag="wp_b")
                for ho in range(NH):
                    for half in range(2):
                        csl = slice(half * (C // 2), (half + 1) * (C // 2))
                        ws = wps.tile([P, C // 2], F32, tag="wpstage", name="ws")
                        nc.sync.dma_start(ws[:], wp[:, ho, csl])
                        nc.vector.tensor_copy(wp_b[:, ho, csl], ws[:])

                with tc.tile_pool(name="psS2", bufs=2, space="PSUM") as psS2, \
                     tc.tile_pool(name="psO", bufs=2, space="PSUM") as psO, \
                     tc.tile_pool(name="psR", bufs=2, space="PSUM") as psR:
                    for h in range(NH):
                        qr = raw[h]
                        kr = raw[4 + h]
                        for qo in range(NQ):
                            qsl = slice(qo * TQ, (qo + 1) * TQ)
                            nfull = 4 * qo
                            nk = nfull + 4
                            ps_o = psO.tile([P, TQ], F32, tag="psout",
                                            name="ps_o")
                            ps_r = psR.tile([1, TQ], F32, tag="psrow",
                                            name="ps_r")
                            # full k-chunks, paired for 1024-wide exps
                            for pr in range(nfull // 2):
                                ps2 = psS2.tile([P, 2, TQ], F32, tag="psscore",
                                                name="ps2")
                                for s in range(2):
                                    j = 2 * pr + s
                                    nc.tensor.matmul(
                                        ps2[:, s, :],
                                        lhsT=kr[:, j * P:(j + 1) * P],
                                        rhs=qr[:, qsl], start=True, stop=True)
                                et2 = etp.tile([P, 2, TQ], BF16, tag="et2",
                                               name="et2")
                                nc.scalar.activation(et2[:], ps2[:], AF.Exp,
                                                     scale=SCALE)
                                for s in range(2):
                                    j = 2 * pr + s
                                    nc.tensor.matmul(
                                        ps_o[:], lhsT=v_b[h][:, j, :],
                                        rhs=et2[:, s, :],
                                        start=(j == 0), stop=False)
                                    nc.tensor.matmul(
                                        ps_r[:], lhsT=ones_sb[:, 0:1],
                                        rhs=et2[:, s, :],
                                        start=(j == 0), stop=False)
                            # 4 diagonal chunks, shrunk to the causal span
                            for jr in range(4):
                                j = nfull + jr
                                off = jr * P
                                w = TQ - off
                                ps2 = psS2.tile([P, 2, TQ], F32, tag="psscore",
                                                name="ps2d")
                                nc.tensor.matmul(
                                    ps2[:, 0, off:TQ],
                                    lhsT=kr[:, j * P:(j + 1) * P],
                                    rhs=qr[:, qo * TQ + off:(qo + 1) * TQ],
                                    start=True, stop=True)
                                et = etp.tile([P, TQ], BF16, tag="et1",
                                              name="et1", bufs=4)
                                nc.scalar.activation(et[:, off:TQ],
                                                     ps2[:, 0, off:TQ], AF.Exp,
                                                     scale=SCALE)
                                nc.vector.tensor_mul(et[:, off:off + P],
                                                     et[:, off:off + P],
                                                     mask_sb[:])
                                last = (jr == 3)
                                nc.tensor.matmul(
                                    ps_o[:, off:TQ], lhsT=v_b[h][:, j, :],
                                    rhs=et[:, off:TQ],
                                    start=(j == 0), stop=last)
                                nc.tensor.matmul(
                                    ps_r[:, off:TQ], lhsT=ones_sb[:, 0:1],
                                    rhs=et[:, off:TQ],
                                    start=(j == 0), stop=last)
                            # normalize: 1/rowsums, broadcast, scale
                            sums = nrm.tile([1, TQ], F32, tag="sums",
                                            name="sums")
                            nc.vector.tensor_copy(sums[:], ps_r[:])
                            recip = nrm.tile([1, TQ], F32, tag="recip",
                                             name="recip")
                            nc.vector.reciprocal_approx_fast(recip[:], sums[:])
                            bcast = nrm.tile([P, TQ], F32, tag="bcast",
                                             name="bcast")
                            nc.gpsimd.partition_broadcast(bcast[:], recip[:])
                            nc.vector.tensor_mul(outT[h][:, qsl], ps_o[:],
                                                 bcast[:])

                # =============== Phase F: projection ===============
                with tc.tile_pool(name="psPj", bufs=6, space="PSUM") as psPj, \
                     tc.tile_pool(name="ystg", bufs=3) as ystg:
                    for t in range(NT):
                        pss = [psPj.tile([P, TQ], F32, tag="psproj",
                                         name="psy") for _ in range(NQ)]
                        for h in range(NH):
                            for cn in range(NQ):
                                nc.tensor.matmul(
                                    pss[cn][:],
                                    lhsT=outT[h][:, t * P:(t + 1) * P],
                                    rhs=wp_b[:, h, cn * TQ:(cn + 1) * TQ],
                                    start=(h == 0), stop=(h == NH - 1))
                        for cn in range(NQ):
                            ys = ystg.tile([P, TQ], F32, tag="ystage",
                                           name="ys")
                            nc.scalar.activation(ys[:], pss[cn][:], AF.Copy)
                            nc.sync.dma_start(
                                y[t * P:(t + 1) * P, cn * TQ:(cn + 1) * TQ],
                                ys[:])

    nc.compile()
    return nc


def _get_nc():
    global _CACHED_NC
    if _CACHED_NC is None:
        _CACHED_NC = build_nc()
    return _CACHED_NC


LAST_RESULTS = None


def kernel(x, cos, sin, W_attn, W_proj):
    global LAST_RESULTS
    x = np.asarray(x, np.float32)
    cos = np.asarray(cos, np.float32)
    sin = np.asarray(sin, np.float32)
    W_attn = np.asarray(W_attn, np.float32)
    W_proj = np.asarray(W_proj, np.float32)
    B = x.shape[0]

    cosT = np.ascontiguousarray(cos.T)                     # [D, T]
    sinT = np.ascontiguousarray(sin.T).copy()
    sinT[: D // 2] *= -1.0                                 # sign-folded rotate_half

    xTs = [np.ascontiguousarray(x[b].T) for b in range(B)]
    in_maps = []
    for b in range(B):
        for g in range(4):
            csl = slice(g * 512, (g + 1) * 512)
            wqk = np.ascontiguousarray(
                np.concatenate([W_attn[:, csl], W_attn[:, C:][:, csl]], axis=1))
            wv = np.ascontiguousarray(W_attn[:, 2 * C:][:, csl])
            wpg = np.ascontiguousarray(W_proj[g * 512:(g + 1) * 512, :])
            in_maps.append({"xT": xTs[b], "wqk": wqk, "wv": wv, "wp": wpg,
                            "cosT": cosT, "sinT": sinT})

    nc = _get_nc()
    res = run_bass_kernel_spmd(nc, in_maps, core_ids=list(range(8)),
                               trace=TRACE)
    LAST_RESULTS = res

    out = np.zeros((B, T, C), np.float32)
    for b in range(B):
        acc = res.results[b * 4 + 0]["y"].astype(np.float32)
        for g in range(1, 4):
            acc = acc + res.results[b * 4 + g]["y"]
        out[b] = acc
    return out


# revision 10
# speedup vs baseline: 1.1689x; 1.1205x over previous
# Multi-head self-attention (B=2, T=2048, C=2048, H=16) on 8 trn2 NeuronCores.
# Sharding: core = (batch b, head-group g) with 4 heads per core.
# Per-core program (Tile framework, bf16 matmuls with fp32 PSUM accumulation):
#   qk^T = W_qk^T @ x^T   (lhsT = W chunks, rhs = x^T)      -> [D, T] per head
#   v    = x @ W_v        (lhsT = x^T chunks, rhs = W_v)    -> [T, D] natural
#   RoPE on q^T/k^T via half-swap DMA + elementwise mul/add (in place)
#   S^T tile = k_rope^T.T @ q_rope^T ; E^T = exp(scale*S^T) (causal)
#   out^T = v.T @ E^T ; rowsums via ones-matmul ; normalize by bcast(1/sums)
#   y_partial = out_heads^T.T @ W_p rows  -> [T, C], host sums 4 partials.
import sys

import numpy as np
import ml_dtypes

for _p in ("/opt/trn_rl_repo",):
    if _p not in sys.path:
        sys.path.append(_p)

import concourse.bass as bass
import concourse.mybir as mybir
import concourse.tile as tile
from concourse import bacc
from concourse.bass_utils import run_bass_kernel_spmd

P = 128
T = 2048
C = 2048
D = 128
NH = 4            # heads per core
KO = C // P       # 16 contraction chunks
TQ = 512          # q-tile width
NQ = T // TQ      # 4
NT = T // P       # 16 t-subtiles
SCALE = float(np.float32(1.0) / np.sqrt(np.float32(D)))

F32 = mybir.dt.float32
BF16 = mybir.dt.bfloat16
AF = mybir.ActivationFunctionType

TRACE = False
_CACHED_NC = None


def _tri_mask_np():
    # keep iff p <= q  (k-row p of the 128-wide diagonal sub-block vs local q)
    p = np.arange(P)[:, None]
    q = np.arange(P)[None, :]
    return (p <= q).astype(ml_dtypes.bfloat16)


def build_nc():
    nc = bacc.Bacc("TRN2", target_bir_lowering=False, debug=False,
                   enable_asserts=False)

    xT_d = nc.dram_tensor("xT", [C, T], F32, kind="ExternalInput")
    wqk_d = nc.dram_tensor("wqk", [C, 2 * NH * D], F32, kind="ExternalInput")
    wv_d = nc.dram_tensor("wv", [C, NH * D], F32, kind="ExternalInput")
    wp_d = nc.dram_tensor("wp", [NH * D, C], F32, kind="ExternalInput")
    cos_d = nc.dram_tensor("cosT", [D, T], F32, kind="ExternalInput")
    sin_d = nc.dram_tensor("sinT", [D, T], F32, kind="ExternalInput")
    y_d = nc.dram_tensor("y", [T, C], F32, kind="ExternalOutput")

    mask_d = nc.inline_tensor(_tri_mask_np(), name="trimask")

    xT = xT_d.ap().rearrange("(ko p) t -> p ko t", p=P)          # [128,16,2048]
    wqk = wqk_d.ap().rearrange("(ko p) m -> p ko m", p=P)        # [128,16,1024]
    wv = wv_d.ap().rearrange("(ko p) m -> p ko m", p=P)          # [128,16,512]
    wp = wp_d.ap().rearrange("(ho p) c -> p ho c", p=P)          # [128,4,2048]
    y = y_d.ap()

    with tile.TileContext(nc) as tc:
        with (
            tc.tile_pool(name="glob", bufs=1) as glob,
            tc.tile_pool(name="rawp", bufs=1) as rawp,
        ):
            ones_sb = glob.tile([P, 1], BF16, tag="ones")
            nc.vector.memset(ones_sb[:], 1.0)
            v_b = [glob.tile([P, NT, P], BF16, tag=f"v_b{h}", name=f"v_b{h}")
                   for h in range(NH)]
            raw = [rawp.tile([P, T], BF16, tag=f"raw{m}", name=f"raw{m}")
                   for m in range(8)]

            # =============== Phase B: qkv matmuls + RoPE ===============
            with tc.tile_pool(name="loadB", bufs=1) as lB, \
                 tc.tile_pool(name="stage", bufs=4) as stg, \
                 tc.tile_pool(name="shufp", bufs=1) as shufp, \
                 tc.tile_pool(name="psB", bufs=6, space="PSUM") as psB:

                # Interleave wqk / x^T loads so matmul m=0 can start after
                # ~3MB of DMA instead of after all of x^T.
                xb = lB.tile([P, KO, T], BF16, tag="xb")
                wqk_b = [lB.tile([P, KO, P], BF16, tag=f"wqk_b{m}",
                                 name=f"wqk_b{m}") for m in range(8)]

                def load_wqk(m):
                    for half in range(2):
                        ksl = slice(half * (KO // 2), (half + 1) * (KO // 2))
                        wst = stg.tile([P, KO // 2, P], F32, tag="stg4k",
                                       name="wst")
                        nc.sync.dma_start(wst[:], wqk[:, ksl, m * P:(m + 1) * P])
                        nc.scalar.activation(wqk_b[m][:, ksl, :], wst[:],
                                             AF.Copy)

                def load_xb(ko):
                    for half in range(2):
                        sl = slice(half * (T // 2), (half + 1) * (T // 2))
                        xs = stg.tile([P, T // 2], F32, tag="stg4k", name="xs")
                        nc.sync.dma_start(xs[:], xT[:, ko, sl])
                        nc.vector.tensor_copy(xb[:, ko, sl], xs[:])

                load_wqk(0)
                load_xb(0)
                for m in range(1, 8):
                    load_wqk(m)
                    load_xb(m)
                for ko in range(8, KO):
                    load_xb(ko)

                # cos/sin -> bf16 (needed by rope, which overlaps qk matmuls)
                cos_b = lB.tile([P, T], BF16, tag="cos_b")
                sin_b = lB.tile([P, T], BF16, tag="sin_b")
                for half in range(2):
                    sl = slice(half * (T // 2), (half + 1) * (T // 2))
                    cs = stg.tile([P, T // 2], F32, tag="stg4k", name="cs")
                    nc.sync.dma_start(cs[:], cos_d.ap()[:, sl])
                    nc.scalar.activation(cos_b[:, sl], cs[:], AF.Copy)
                    ss = stg.tile([P, T // 2], F32, tag="stg4k", name="ss")
                    nc.sync.dma_start(ss[:], sin_d.ap()[:, sl])
                    nc.scalar.activation(sin_b[:, sl], ss[:], AF.Copy)

                # qk matmuls, LDWEIGHTS amortized over the 4 T-tiles;
                # rope each raw[m] as soon as its evacuations are done.
                for m in range(8):
                    pss = [psB.tile([P, TQ], F32, tag="psBig", name="psqk")
                           for _ in range(NQ)]
                    for ko in range(KO):
                        for n in range(NQ):
                            nc.tensor.matmul(
                                pss[n][:], lhsT=wqk_b[m][:, ko, :],
                                rhs=xb[:, ko, n * TQ:(n + 1) * TQ],
                                start=(ko == 0), stop=(ko == KO - 1))
                    for n in range(NQ):
                        nc.scalar.activation(
                            raw[m][:, n * TQ:(n + 1) * TQ], pss[n][:], AF.Copy)
                    # rope in place
                    r = raw[m]
                    sh = shufp.tile([P, T], BF16, tag="shuf", name="sh")
                    nc.sync.dma_start(sh[0:64, :], r[64:128, :])
                    nc.sync.dma_start(sh[64:128, :], r[0:64, :])
                    nc.vector.tensor_mul(sh[:], sh[:], sin_b[:])
                    nc.vector.tensor_mul(r[:], r[:], cos_b[:])
                    nc.vector.tensor_add(r[:], r[:], sh[:])

                # v weights + v matmuls (natural layout)
                wv_b = lB.tile([P, KO, NH * D], BF16, tag="wv_b")
                for ko in range(KO):
                    wvs = stg.tile([P, NH * D], F32, tag="stg4k", name="wvs")
                    nc.sync.dma_start(wvs[:], wv[:, ko, :])
                    nc.scalar.activation(wv_b[:, ko, :], wvs[:], AF.Copy)
                for t in range(NT):
                    psv = psB.tile([P, TQ], F32, tag="psBig", name="psv")
                    for ko in range(KO):
                        nc.tensor.matmul(
                            psv[:], lhsT=xb[:, ko, t * P:(t + 1) * P],
                            rhs=wv_b[:, ko, :],
                            start=(ko == 0), stop=(ko == KO - 1))
                    for h in range(NH):
                        nc.scalar.activation(
                            v_b[h][:, t, :], psv[:, h * P:(h + 1) * P], AF.Copy)

            # =============== Phase D: attention ===============
            with tc.tile_pool(name="attnp", bufs=1) as ap_, \
                 tc.tile_pool(name="etp", bufs=3) as etp, \
                 tc.tile_pool(name="nrm", bufs=2) as nrm, \
                 tc.tile_pool(name="wps", bufs=2) as wps:

                mask_sb = ap_.tile([P, P], BF16, tag="trimask")
                nc.sync.dma_start(mask_sb[:], mask_d.ap())
                outT = [ap_.tile([P, T], BF16, tag=f"outT{h}", name=f"outT{h}")
                        for h in range(NH)]
                # load wp during attention (cast on vector - ACT is busy)
                wp_b = ap_.tile([P, NH, C], BF16, tag="wp_b")
                for ho in range(NH):
                    for half in range(2):
                        csl = slice(half * (C // 2), (half + 1) * (C // 2))
                        ws = wps.tile([P, C // 2], F32, tag="wpstage", name="ws")
                        nc.sync.dma_start(ws[:], wp[:, ho, csl])
                        nc.vector.tensor_copy(wp_b[:, ho, csl], ws[:])

                with tc.tile_pool(name="psS2", bufs=2, space="PSUM") as psS2, \
                     tc.tile_pool(name="psO", bufs=2, space="PSUM") as psO, \
                     tc.tile_pool(name="psR", bufs=2, space="PSUM") as psR:
                    for h in range(NH):
                        qr = raw[h]
                        kr = raw[4 + h]
                        for qo in range(NQ):
                            qsl = slice(qo * TQ, (qo + 1) * TQ)
                            nfull = 4 * qo
                            nk = nfull + 4
                            ps_o = psO.tile([P, TQ], F32, tag="psout",
                                            name="ps_o")
                            ps_r = psR.tile([1, TQ], F32, tag="psrow",
                                            name="ps_r")
                            # full k-chunks, paired for 1024-wide exps
                            for pr in range(nfull // 2):
                                ps2 = psS2.tile([P, 2, TQ], F32, tag="psscore",
                                                name="ps2")
                                for s in range(2):
                                    j = 2 * pr + s
                                    nc.tensor.matmul(
                                        ps2[:, s, :],
                                        lhsT=kr[:, j * P:(j + 1) * P],
                                        rhs=qr[:, qsl], start=True, stop=True)
                                et2 = etp.tile([P, 2, TQ], BF16, tag="et2",
                                               name="et2", bufs=4)
                                nc.scalar.activation(et2[:], ps2[:], AF.Exp,
                                                     scale=SCALE)
                                for s in range(2):
                                    j = 2 * pr + s
                                    nc.tensor.matmul(
                                        ps_o[:], lhsT=v_b[h][:, j, :],
                                        rhs=et2[:, s, :],
                                        start=(j == 0), stop=False)
                                # one rowsum matmul per pair on the DVE-summed
                                # pair (halves the PE ones-matmul count)
                                esum = etp.tile([P, TQ], BF16, tag="esum",
                                                name="esum", bufs=3)
                                nc.vector.tensor_add(esum[:], et2[:, 0, :],
                                                     et2[:, 1, :])
                                nc.tensor.matmul(
                                    ps_r[:], lhsT=ones_sb[:, 0:1],
                                    rhs=esum[:],
                                    start=(pr == 0), stop=False)
                            # 4 diagonal chunks, shrunk to the causal span
                            for jr in range(4):
                                j = nfull + jr
                                off = jr * P
                                w = TQ - off
                                ps2 = psS2.tile([P, 2, TQ], F32, tag="psscore",
                                                name="ps2d")
                                nc.tensor.matmul(
                                    ps2[:, 0, off:TQ],
                                    lhsT=kr[:, j * P:(j + 1) * P],
                                    rhs=qr[:, qo * TQ + off:(qo + 1) * TQ],
                                    start=True, stop=True)
                                et = etp.tile([P, TQ], BF16, tag="et1",
                                              name="et1", bufs=4)
                                nc.scalar.activation(et[:, off:TQ],
                                                     ps2[:, 0, off:TQ], AF.Exp,
                                                     scale=SCALE)
                                nc.vector.tensor_mul(et[:, off:off + P],
                                                     et[:, off:off + P],
                                                     mask_sb[:])
                                last = (jr == 3)
                                nc.tensor.matmul(
                                    ps_o[:, off:TQ], lhsT=v_b[h][:, j, :],
                                    rhs=et[:, off:TQ],
                                    start=(j == 0), stop=last)
                                nc.tensor.matmul(
                                    ps_r[:, off:TQ], lhsT=ones_sb[:, 0:1],
                                    rhs=et[:, off:TQ],
                                    start=(j == 0), stop=last)
                            # normalize: 1/rowsums, broadcast, scale
                            sums = nrm.tile([1, TQ], F32, tag="sums",
                                            name="sums")
                            nc.vector.tensor_copy(sums[:], ps_r[:])
                            recip = nrm.tile([1, TQ], F32, tag="recip",
                                             name="recip")
                            nc.vector.reciprocal_approx_fast(recip[:], sums[:])
                            bcast = nrm.tile([P, TQ], F32, tag="bcast",
                                             name="bcast")
                            nc.gpsimd.partition_broadcast(bcast[:], recip[:])
                            nc.vector.tensor_mul(outT[h][:, qsl], ps_o[:],
                                                 bcast[:])

                # =============== Phase F: projection ===============
                with tc.tile_pool(name="psPj", bufs=6, space="PSUM") as psPj, \
                     tc.tile_pool(name="ystg", bufs=3) as ystg:
                    for t in range(NT):
                        pss = [psPj.tile([P, TQ], F32, tag="psproj",
                                         name="psy") for _ in range(NQ)]
                        for h in range(NH):
                            for cn in range(NQ):
                                nc.tensor.matmul(
                                    pss[cn][:],
                                    lhsT=outT[h][:, t * P:(t + 1) * P],
                                    rhs=wp_b[:, h, cn * TQ:(cn + 1) * TQ],
                                    start=(h == 0), stop=(h == NH - 1))
                        # stage a full 2048-wide row so the store DMA moves
                        # 8KB contiguous per partition (write-BW friendly)
                        ys = ystg.tile([P, T], F32, tag="ystage", name="ys")
                        for cn in range(NQ):
                            nc.scalar.activation(
                                ys[:, cn * TQ:(cn + 1) * TQ], pss[cn][:],
                                AF.Copy)
                        nc.sync.dma_start(y[t * P:(t + 1) * P, :], ys[:])

    nc.compile()
    return nc


def _get_nc():
    global _CACHED_NC
    if _CACHED_NC is None:
        _CACHED_NC = build_nc()
    return _CACHED_NC


LAST_RESULTS = None


def kernel(x, cos, sin, W_attn, W_proj):
    global LAST_RESULTS
    x = np.asarray(x, np.float32)
    cos = np.asarray(cos, np.float32)
    sin = np.asarray(sin, np.float32)
    W_attn = np.asarray(W_attn, np.float32)
    W_proj = np.asarray(W_proj, np.float32)
    B = x.shape[0]

    cosT = np.ascontiguousarray(cos.T)                     # [D, T]
    sinT = np.ascontiguousarray(sin.T).copy()
    sinT[: D // 2] *= -1.0                                 # sign-folded rotate_half

    xTs = [np.ascontiguousarray(x[b].T) for b in range(B)]
    in_maps = []
    for b in range(B):
        for g in range(4):
            csl = slice(g * 512, (g + 1) * 512)
            wqk = np.ascontiguousarray(
                np.concatenate([W_attn[:, csl], W_attn[:, C:][:, csl]], axis=1))
            wv = np.ascontiguousarray(W_attn[:, 2 * C:][:, csl])
            wpg = np.ascontiguousarray(W_proj[g * 512:(g + 1) * 512, :])
            in_maps.append({"xT": xTs[b], "wqk": wqk, "wv": wv, "wp": wpg,
                            "cosT": cosT, "sinT": sinT})

    nc = _get_nc()
    res = run_bass_kernel_spmd(nc, in_maps, core_ids=list(range(8)),
                               trace=TRACE)
    LAST_RESULTS = res

    out = np.zeros((B, T, C), np.float32)
    for b in range(B):
        acc = res.results[b * 4 + 0]["y"].astype(np.float32)
        for g in range(1, 4):
            acc = acc + res.results[b * 4 + g]["y"]
        out[b] = acc
    return out


# revision 11
# speedup vs baseline: 1.3449x; 1.1506x over previous
# Multi-head self-attention (B=2, T=2048, C=2048, H=16) on 8 trn2 NeuronCores.
# Sharding: core = (batch b, head-group g) with 4 heads per core.
# Inputs are pre-cast to bf16 and packed DMA-friendly on the host (the device
# would do the identical round-to-nearest cast before its bf16 matmuls).
# Per-core program (Tile framework, bf16 matmuls with fp32 PSUM accumulation):
#   qk^T = W_qk^T @ x^T   (lhsT = W chunks, rhs = x^T)      -> [D, T] per head
#   v    = x @ W_v        (lhsT = x^T chunks, rhs = W_v)    -> [T, D] natural
#   RoPE on q^T/k^T via half-swap DMA + elementwise mul/add (in place)
#   S^T tile = k_rope^T.T @ q_rope^T ; E^T = exp(scale*S^T) (causal)
#   out^T = v.T @ E^T ; rowsums via ones-matmul on DVE-paired E tiles
#   normalize off the PSUM path: evac unscaled, scale by bcast(1/sums) in SBUF
#   y_partial = out_heads^T.T @ W_p rows  -> [T, C], host sums 4 partials.
import sys

import numpy as np
import ml_dtypes

for _p in ("/opt/trn_rl_repo",):
    if _p not in sys.path:
        sys.path.append(_p)

import concourse.bass as bass
import concourse.mybir as mybir
import concourse.tile as tile
from concourse import bacc
from concourse.bass_utils import run_bass_kernel_spmd

P = 128
T = 2048
C = 2048
D = 128
NH = 4            # heads per core
KO = C // P       # 16 contraction chunks
TQ = 512          # q-tile width
NQ = T // TQ      # 4
NT = T // P       # 16 t-subtiles
SCALE = float(np.float32(1.0) / np.sqrt(np.float32(D)))

F32 = mybir.dt.float32
BF16 = mybir.dt.bfloat16
AF = mybir.ActivationFunctionType
BF = ml_dtypes.bfloat16

TRACE = False
_CACHED_NC = None


def _tri_mask_np():
    p = np.arange(P)[:, None]
    q = np.arange(P)[None, :]
    return (p <= q).astype(BF)


def build_nc():
    nc = bacc.Bacc("TRN2", target_bir_lowering=False, debug=False,
                   enable_asserts=False)

    # bf16 inputs, packed so every DMA moves >=4KB contiguous per partition
    xT_d = nc.dram_tensor("xT", [C, T], BF16, kind="ExternalInput")
    wqk_d = nc.dram_tensor("wqk", [8, P, KO, P], BF16, kind="ExternalInput")
    wv_d = nc.dram_tensor("wv", [P, KO, NH * D], BF16, kind="ExternalInput")
    wp_d = nc.dram_tensor("wp", [P, NH, C], BF16, kind="ExternalInput")
    cos_d = nc.dram_tensor("cosT", [D, T], BF16, kind="ExternalInput")
    sin_d = nc.dram_tensor("sinT", [D, T], BF16, kind="ExternalInput")
    y_d = nc.dram_tensor("y", [T, C], F32, kind="ExternalOutput")

    mask_d = nc.inline_tensor(_tri_mask_np(), name="trimask")

    xT = xT_d.ap().rearrange("(ko p) t -> p ko t", p=P)          # [128,16,2048]
    wqk = wqk_d.ap()
    y = y_d.ap()

    with tile.TileContext(nc) as tc:
        with (
            tc.tile_pool(name="glob", bufs=1) as glob,
            tc.tile_pool(name="rawp", bufs=1) as rawp,
        ):
            ones_sb = glob.tile([P, 1], BF16, tag="ones")
            nc.vector.memset(ones_sb[:], 1.0)
            v_b = [glob.tile([P, NT, P], BF16, tag=f"v_b{h}", name=f"v_b{h}")
                   for h in range(NH)]
            raw = [rawp.tile([P, T], BF16, tag=f"raw{m}", name=f"raw{m}")
                   for m in range(8)]

            # =============== Phase B: qkv matmuls + RoPE ===============
            with tc.tile_pool(name="loadB", bufs=1) as lB, \
                 tc.tile_pool(name="shufp", bufs=1) as shufp, \
                 tc.tile_pool(name="psB", bufs=6, space="PSUM") as psB:

                xb = lB.tile([P, KO, T], BF16, tag="xb")
                wqk_b = [lB.tile([P, KO, P], BF16, tag=f"wqk_b{m}",
                                 name=f"wqk_b{m}") for m in range(8)]
                # interleave weight/x loads; arrival order matches use order
                for m in range(8):
                    nc.sync.dma_start(wqk_b[m][:], wqk[m])
                    nc.sync.dma_start(xb[:, m, :], xT[:, m, :])
                for ko in range(8, KO):
                    nc.sync.dma_start(xb[:, ko, :], xT[:, ko, :])
                cos_b = lB.tile([P, T], BF16, tag="cos_b")
                sin_b = lB.tile([P, T], BF16, tag="sin_b")
                nc.sync.dma_start(cos_b[:], cos_d.ap())
                nc.sync.dma_start(sin_b[:], sin_d.ap())
                wv_b = lB.tile([P, KO, NH * D], BF16, tag="wv_b")
                nc.sync.dma_start(wv_b[:], wv_d.ap())

                # qk matmuls; k-order staggered by m so each group consumes
                # chunks roughly in DMA-arrival order
                for m in range(8):
                    pss = [psB.tile([P, TQ], F32, tag="psBig", name="psqk")
                           for _ in range(NQ)]
                    kos = [(2 * m + i) % KO for i in range(KO)]
                    for i, ko in enumerate(kos):
                        for n in range(NQ):
                            nc.tensor.matmul(
                                pss[n][:], lhsT=wqk_b[m][:, ko, :],
                                rhs=xb[:, ko, n * TQ:(n + 1) * TQ],
                                start=(i == 0), stop=(i == KO - 1))
                    for n in range(NQ):
                        nc.scalar.activation(
                            raw[m][:, n * TQ:(n + 1) * TQ], pss[n][:], AF.Copy)
                    # rope in place
                    r = raw[m]
                    sh = shufp.tile([P, T], BF16, tag="shuf", name="sh")
                    nc.sync.dma_start(sh[0:64, :], r[64:128, :])
                    nc.sync.dma_start(sh[64:128, :], r[0:64, :])
                    nc.vector.tensor_mul(sh[:], sh[:], sin_b[:])
                    nc.vector.tensor_mul(r[:], r[:], cos_b[:])
                    nc.vector.tensor_add(r[:], r[:], sh[:])

                # v matmuls (natural layout)
                for t in range(NT):
                    psv = psB.tile([P, TQ], F32, tag="psBig", name="psv")
                    for ko in range(KO):
                        nc.tensor.matmul(
                            psv[:], lhsT=xb[:, ko, t * P:(t + 1) * P],
                            rhs=wv_b[:, ko, :],
                            start=(ko == 0), stop=(ko == KO - 1))
                    for h in range(NH):
                        nc.scalar.activation(
                            v_b[h][:, t, :], psv[:, h * P:(h + 1) * P], AF.Copy)

            # =============== Phase D: attention ===============
            with tc.tile_pool(name="attnp", bufs=1) as ap_, \
                 tc.tile_pool(name="etp", bufs=3) as etp, \
                 tc.tile_pool(name="nrm", bufs=2) as nrm:

                mask_sb = ap_.tile([P, P], BF16, tag="trimask")
                nc.sync.dma_start(mask_sb[:], mask_d.ap())
                outT = [ap_.tile([P, T], BF16, tag=f"outT{h}", name=f"outT{h}")
                        for h in range(NH)]
                wp_b = ap_.tile([P, NH, C], BF16, tag="wp_b")
                nc.sync.dma_start(wp_b[:], wp_d.ap())

                with tc.tile_pool(name="psS2", bufs=2, space="PSUM") as psS2, \
                     tc.tile_pool(name="psO", bufs=2, space="PSUM") as psO, \
                     tc.tile_pool(name="psR", bufs=2, space="PSUM") as psR:
                    for h in range(NH):
                        qr = raw[h]
                        kr = raw[4 + h]
                        for qo in range(NQ):
                            qsl = slice(qo * TQ, (qo + 1) * TQ)
                            nfull = 4 * qo
                            ps_o = psO.tile([P, TQ], F32, tag="psout",
                                            name="ps_o")
                            ps_r = psR.tile([1, TQ], F32, tag="psrow",
                                            name="ps_r")
                            for pr in range(nfull // 2):
                                ps2 = psS2.tile([P, 2, TQ], F32, tag="psscore",
                                                name="ps2")
                                for s in range(2):
                                    j = 2 * pr + s
                                    nc.tensor.matmul(
                                        ps2[:, s, :],
                                        lhsT=kr[:, j * P:(j + 1) * P],
                                        rhs=qr[:, qsl], start=True, stop=True)
                                et2 = etp.tile([P, 2, TQ], BF16, tag="et2",
                                               name="et2", bufs=4)
                                nc.scalar.activation(et2[:], ps2[:], AF.Exp,
                                                     scale=SCALE)
                                for s in range(2):
                                    j = 2 * pr + s
                                    nc.tensor.matmul(
                                        ps_o[:], lhsT=v_b[h][:, j, :],
                                        rhs=et2[:, s, :],
                                        start=(j == 0), stop=False)
                                esum = etp.tile([P, TQ], BF16, tag="esum",
                                                name="esum", bufs=3)
                                nc.vector.tensor_add(esum[:], et2[:, 0, :],
                                                     et2[:, 1, :])
                                nc.tensor.matmul(
                                    ps_r[:], lhsT=ones_sb[:, 0:1],
                                    rhs=esum[:],
                                    start=(pr == 0), stop=False)
                            for jr in range(4):
                                j = nfull + jr
                                off = jr * P
                                ps2 = psS2.tile([P, 2, TQ], F32, tag="psscore",
                                                name="ps2d")
                                nc.tensor.matmul(
                                    ps2[:, 0, off:TQ],
                                    lhsT=kr[:, j * P:(j + 1) * P],
                                    rhs=qr[:, qo * TQ + off:(qo + 1) * TQ],
                                    start=True, stop=True)
                                et = etp.tile([P, TQ], BF16, tag="et1",
                                              name="et1", bufs=4)
                                nc.scalar.activation(et[:, off:TQ],
                                                     ps2[:, 0, off:TQ], AF.Exp,
                                                     scale=SCALE)
                                nc.vector.tensor_mul(et[:, off:off + P],
                                                     et[:, off:off + P],
                                                     mask_sb[:])
                                last = (jr == 3)
                                nc.tensor.matmul(
                                    ps_o[:, off:TQ], lhsT=v_b[h][:, j, :],
                                    rhs=et[:, off:TQ],
                                    start=(j == 0), stop=last)
                                nc.tensor.matmul(
                                    ps_r[:, off:TQ], lhsT=ones_sb[:, 0:1],
                                    rhs=et[:, off:TQ],
                                    start=(j == 0), stop=last)
                            # evacuate unscaled output now (frees the PSUM
                            # bank fast); normalize later in SBUF, off the
                            # PE critical path
                            outU = nrm.tile([P, TQ], F32, tag="outU",
                                            name="outU", bufs=3)
                            nc.scalar.activation(outU[:], ps_o[:], AF.Copy)
                            sums = nrm.tile([1, TQ], F32, tag="sums",
                                            name="sums")
                            nc.vector.tensor_copy(sums[:], ps_r[:])
                            recip = nrm.tile([1, TQ], F32, tag="recip",
                                             name="recip")
                            nc.vector.reciprocal_approx_fast(recip[:], sums[:])
                            bcast = nrm.tile([P, TQ], F32, tag="bcast",
                                             name="bcast")
                            nc.gpsimd.partition_broadcast(bcast[:], recip[:])
                            nc.vector.tensor_mul(outT[h][:, qsl], outU[:],
                                                 bcast[:])

                # =============== Phase F: projection ===============
                with tc.tile_pool(name="psPj", bufs=6, space="PSUM") as psPj, \
                     tc.tile_pool(name="ystg", bufs=3) as ystg:
                    for t in range(NT):
                        pss = [psPj.tile([P, TQ], F32, tag="psproj",
                                         name="psy") for _ in range(NQ)]
                        for h in range(NH):
                            for cn in range(NQ):
                                nc.tensor.matmul(
                                    pss[cn][:],
                                    lhsT=outT[h][:, t * P:(t + 1) * P],
                                    rhs=wp_b[:, h, cn * TQ:(cn + 1) * TQ],
                                    start=(h == 0), stop=(h == NH - 1))
                        ys = ystg.tile([P, T], F32, tag="ystage", name="ys")
                        for cn in range(NQ):
                            nc.scalar.activation(
                                ys[:, cn * TQ:(cn + 1) * TQ], pss[cn][:],
                                AF.Copy)
                        eng = nc.sync if t % 2 == 0 else nc.scalar
                        eng.dma_start(y[t * P:(t + 1) * P, :], ys[:])

    nc.compile()
    return nc


def _get_nc():
    global _CACHED_NC
    if _CACHED_NC is None:
        _CACHED_NC = build_nc()
    return _CACHED_NC


LAST_RESULTS = None


def kernel(x, cos, sin, W_attn, W_proj):
    global LAST_RESULTS
    x = np.asarray(x, np.float32)
    cos = np.asarray(cos, np.float32)
    sin = np.asarray(sin, np.float32)
    W_attn = np.asarray(W_attn, np.float32)
    W_proj = np.asarray(W_proj, np.float32)
    B = x.shape[0]

    cosT = np.ascontiguousarray(cos.T).astype(BF)          # [D, T]
    sinTf = np.ascontiguousarray(sin.T).copy()
    sinTf[: D // 2] *= -1.0                                # sign-folded rotate
    sinT = sinTf.astype(BF)

    xTs = [np.ascontiguousarray(x[b].T).astype(BF) for b in range(B)]
    in_maps = []
    for b in range(B):
        for g in range(4):
            csl = slice(g * 512, (g + 1) * 512)
            wqk2 = np.concatenate([W_attn[:, csl], W_attn[:, C:][:, csl]],
                                  axis=1).astype(BF)       # [C, 1024]
            # pack [8, 128, 16, 128]: wqkr[m, p, ko, j] = wqk2[128*ko+p, 128*m+j]
            wqkr = np.ascontiguousarray(
                wqk2.reshape(KO, P, 8, P).transpose(2, 1, 0, 3))
            wv2 = W_attn[:, 2 * C:][:, csl].astype(BF)     # [C, 512]
            wvr = np.ascontiguousarray(
                wv2.reshape(KO, P, NH * D).transpose(1, 0, 2))  # [128,16,512]
            wp2 = W_proj[g * 512:(g + 1) * 512, :].astype(BF)   # [512, C]
            wpr = np.ascontiguousarray(
                wp2.reshape(NH, P, C).transpose(1, 0, 2))       # [128,4,2048]
            in_maps.append({"xT": xTs[b], "wqk": wqkr, "wv": wvr, "wp": wpr,
                            "cosT": cosT, "sinT": sinT})

    nc = _get_nc()
    res = run_bass_kernel_spmd(nc, in_maps, core_ids=list(range(8)),
                               trace=TRACE)
    LAST_RESULTS = res

    out = np.zeros((B, T, C), np.float32)
    for b in range(B):
        acc = res.results[b * 4 + 0]["y"].astype(np.float32)
        for g in range(1, 4):
            acc = acc + res.results[b * 4 + g]["y"]
        out[b] = acc
    return out


# revision 12
# speedup vs baseline: 1.3793x; 1.0256x over previous
# Multi-head self-attention (B=2, T=2048, C=2048, H=16) on 8 trn2 NeuronCores.
# Sharding: core = (batch b, head-group g) with 4 heads per core.
# Inputs are pre-cast to bf16 and packed DMA-friendly on the host (the device
# would do the identical round-to-nearest cast before its bf16 matmuls).
# Per-core program (Tile framework, bf16 matmuls with fp32 PSUM accumulation):
#   qk^T = W_qk^T @ x^T   (lhsT = W chunks, rhs = x^T)      -> [D, T] per head
#   v    = x @ W_v        (lhsT = x^T chunks, rhs = W_v)    -> [T, D] natural
#   RoPE on q^T/k^T via half-swap DMA + elementwise mul/add (in place)
#   S^T tile = k_rope^T.T @ q_rope^T ; E^T = exp(scale*S^T) (causal)
#   out^T = v.T @ E^T ; rowsums via ones-matmul on DVE-paired E tiles
#   normalize off the PSUM path: evac unscaled, scale by bcast(1/sums) in SBUF
#   y_partial = out_heads^T.T @ W_p rows  -> [T, C], host sums 4 partials.
import sys

import numpy as np
import ml_dtypes

for _p in ("/opt/trn_rl_repo",):
    if _p not in sys.path:
        sys.path.append(_p)

import concourse.bass as bass
import concourse.mybir as mybir
import concourse.tile as tile
from concourse import bacc
from concourse.bass_utils import run_bass_kernel_spmd

P = 128
T = 2048
C = 2048
D = 128
NH = 4            # heads per core
KO = C // P       # 16 contraction chunks
TQ = 512          # q-tile width
NQ = T // TQ      # 4
NT = T // P       # 16 t-subtiles
SCALE = float(np.float32(1.0) / np.sqrt(np.float32(D)))

F32 = mybir.dt.float32
BF16 = mybir.dt.bfloat16
AF = mybir.ActivationFunctionType
BF = ml_dtypes.bfloat16

TRACE = False
_CACHED_NC = None


def _tri_mask_np():
    p = np.arange(P)[:, None]
    q = np.arange(P)[None, :]
    return (p <= q).astype(BF)


def build_nc():
    nc = bacc.Bacc("TRN2", target_bir_lowering=False, debug=False,
                   enable_asserts=False)

    # bf16 inputs, packed so every DMA moves >=4KB contiguous per partition
    xT_d = nc.dram_tensor("xT", [C, T], BF16, kind="ExternalInput")
    wqk_d = nc.dram_tensor("wqk", [8, P, KO, P], BF16, kind="ExternalInput")
    wv_d = nc.dram_tensor("wv", [P, KO, NH * D], BF16, kind="ExternalInput")
    wp_d = nc.dram_tensor("wp", [P, NH, C], BF16, kind="ExternalInput")
    cos_d = nc.dram_tensor("cosT", [D, T], BF16, kind="ExternalInput")
    sin_d = nc.dram_tensor("sinT", [D, T], BF16, kind="ExternalInput")
    y_d = nc.dram_tensor("y", [T, C], F32, kind="ExternalOutput")

    mask_d = nc.inline_tensor(_tri_mask_np(), name="trimask")

    xT = xT_d.ap().rearrange("(ko p) t -> p ko t", p=P)          # [128,16,2048]
    wqk = wqk_d.ap()
    y = y_d.ap()

    with tile.TileContext(nc) as tc:
        with (
            tc.tile_pool(name="glob", bufs=1) as glob,
            tc.tile_pool(name="rawp", bufs=1) as rawp,
        ):
            ones_sb = glob.tile([P, 1], BF16, tag="ones")
            nc.vector.memset(ones_sb[:], 1.0)
            v_b = [glob.tile([P, NT, P], BF16, tag=f"v_b{h}", name=f"v_b{h}")
                   for h in range(NH)]
            raw = [rawp.tile([P, T], BF16, tag=f"raw{m}", name=f"raw{m}")
                   for m in range(8)]

            # =============== Phase B: qkv matmuls + RoPE ===============
            with tc.tile_pool(name="loadB", bufs=1) as lB, \
                 tc.tile_pool(name="shufp", bufs=1) as shufp, \
                 tc.tile_pool(name="psB", bufs=6, space="PSUM") as psB:

                xb = lB.tile([P, KO, T], BF16, tag="xb")
                wqk_b = [lB.tile([P, KO, P], BF16, tag=f"wqk_b{m}",
                                 name=f"wqk_b{m}") for m in range(8)]
                # interleave weight/x loads; arrival order matches use order
                for m in range(8):
                    nc.sync.dma_start(wqk_b[m][:], wqk[m])
                    nc.sync.dma_start(xb[:, m, :], xT[:, m, :])
                for ko in range(8, KO):
                    nc.sync.dma_start(xb[:, ko, :], xT[:, ko, :])
                cos_b = lB.tile([P, T], BF16, tag="cos_b")
                sin_b = lB.tile([P, T], BF16, tag="sin_b")
                nc.sync.dma_start(cos_b[:], cos_d.ap())
                nc.sync.dma_start(sin_b[:], sin_d.ap())
                wv_b = lB.tile([P, KO, NH * D], BF16, tag="wv_b")
                nc.sync.dma_start(wv_b[:], wv_d.ap())

                # qk matmuls; k-order staggered by m so each group consumes
                # chunks roughly in DMA-arrival order
                for m in range(8):
                    pss = [psB.tile([P, TQ], F32, tag="psBig", name="psqk")
                           for _ in range(NQ)]
                    kos = [(2 * m + i) % KO for i in range(KO)]
                    for i, ko in enumerate(kos):
                        for n in range(NQ):
                            nc.tensor.matmul(
                                pss[n][:], lhsT=wqk_b[m][:, ko, :],
                                rhs=xb[:, ko, n * TQ:(n + 1) * TQ],
                                start=(i == 0), stop=(i == KO - 1))
                    for n in range(NQ):
                        nc.scalar.activation(
                            raw[m][:, n * TQ:(n + 1) * TQ], pss[n][:], AF.Copy)
                    # rope in place
                    r = raw[m]
                    sh = shufp.tile([P, T], BF16, tag="shuf", name="sh")
                    nc.sync.dma_start(sh[0:64, :], r[64:128, :])
                    nc.sync.dma_start(sh[64:128, :], r[0:64, :])
                    nc.vector.tensor_mul(sh[:], sh[:], sin_b[:])
                    nc.vector.tensor_mul(r[:], r[:], cos_b[:])
                    nc.vector.tensor_add(r[:], r[:], sh[:])

                # v matmuls (natural layout)
                for t in range(NT):
                    psv = psB.tile([P, TQ], F32, tag="psBig", name="psv")
                    for ko in range(KO):
                        nc.tensor.matmul(
                            psv[:], lhsT=xb[:, ko, t * P:(t + 1) * P],
                            rhs=wv_b[:, ko, :],
                            start=(ko == 0), stop=(ko == KO - 1))
                    for h in range(NH):
                        nc.scalar.activation(
                            v_b[h][:, t, :], psv[:, h * P:(h + 1) * P], AF.Copy)

            # =============== Phase D: attention ===============
            with tc.tile_pool(name="attnp", bufs=1) as ap_, \
                 tc.tile_pool(name="etp", bufs=3) as etp, \
                 tc.tile_pool(name="nrm", bufs=2) as nrm:

                mask_sb = ap_.tile([P, P], BF16, tag="trimask")
                nc.sync.dma_start(mask_sb[:], mask_d.ap())
                outT = [ap_.tile([P, T], BF16, tag=f"outT{h}", name=f"outT{h}")
                        for h in range(NH)]
                wp_b = ap_.tile([P, NH, C], BF16, tag="wp_b")
                nc.sync.dma_start(wp_b[:], wp_d.ap())

                with tc.tile_pool(name="psS2", bufs=2, space="PSUM") as psS2, \
                     tc.tile_pool(name="psO", bufs=2, space="PSUM") as psO, \
                     tc.tile_pool(name="psR", bufs=2, space="PSUM") as psR:
                    for h in range(NH):
                        qr = raw[h]
                        kr = raw[4 + h]
                        # qo descending: dense large-qo tiles first, so the
                        # latency-bound qo=0 chain overlaps other work
                        for qo in reversed(range(NQ)):
                            qsl = slice(qo * TQ, (qo + 1) * TQ)
                            nfull = 4 * qo
                            ps_o = psO.tile([P, TQ], F32, tag="psout",
                                            name="ps_o")
                            ps_r = psR.tile([1, TQ], F32, tag="psrow",
                                            name="ps_r")
                            # diagonal chunks first: their exp/mask latency
                            # chains hide under the dense pairs that follow
                            for jr in range(4):
                                j = nfull + jr
                                off = jr * P
                                ps2 = psS2.tile([P, 2, TQ], F32, tag="psscore",
                                                name="ps2d")
                                nc.tensor.matmul(
                                    ps2[:, 0, off:TQ],
                                    lhsT=kr[:, j * P:(j + 1) * P],
                                    rhs=qr[:, qo * TQ + off:(qo + 1) * TQ],
                                    start=True, stop=True)
                                et = etp.tile([P, TQ], BF16, tag="et1",
                                              name="et1", bufs=4)
                                nc.scalar.activation(et[:, off:TQ],
                                                     ps2[:, 0, off:TQ], AF.Exp,
                                                     scale=SCALE)
                                nc.vector.tensor_mul(et[:, off:off + P],
                                                     et[:, off:off + P],
                                                     mask_sb[:])
                                last = (jr == 3) and nfull == 0
                                nc.tensor.matmul(
                                    ps_o[:, off:TQ], lhsT=v_b[h][:, j, :],
                                    rhs=et[:, off:TQ],
                                    start=(jr == 0), stop=last)
                                nc.tensor.matmul(
                                    ps_r[:, off:TQ], lhsT=ones_sb[:, 0:1],
                                    rhs=et[:, off:TQ],
                                    start=(jr == 0), stop=last)
                            for pr in range(nfull // 2):
                                ps2 = psS2.tile([P, 2, TQ], F32, tag="psscore",
                                                name="ps2")
                                for s in range(2):
                                    j = 2 * pr + s
                                    nc.tensor.matmul(
                                        ps2[:, s, :],
                                        lhsT=kr[:, j * P:(j + 1) * P],
                                        rhs=qr[:, qsl], start=True, stop=True)
                                et2 = etp.tile([P, 2, TQ], BF16, tag="et2",
                                               name="et2", bufs=4)
                                nc.scalar.activation(et2[:], ps2[:], AF.Exp,
                                                     scale=SCALE)
                                last = (pr == nfull // 2 - 1)
                                for s in range(2):
                                    j = 2 * pr + s
                                    nc.tensor.matmul(
                                        ps_o[:], lhsT=v_b[h][:, j, :],
                                        rhs=et2[:, s, :],
                                        start=False, stop=(last and s == 1))
                                esum = etp.tile([P, TQ], BF16, tag="esum",
                                                name="esum", bufs=3)
                                nc.vector.tensor_add(esum[:], et2[:, 0, :],
                                                     et2[:, 1, :])
                                nc.tensor.matmul(
                                    ps_r[:], lhsT=ones_sb[:, 0:1],
                                    rhs=esum[:],
                                    start=False, stop=last)
                            # evacuate unscaled output now (frees the PSUM
                            # bank fast); normalize later in SBUF, off the
                            # PE critical path
                            outU = nrm.tile([P, TQ], F32, tag="outU",
                                            name="outU", bufs=3)
                            nc.scalar.activation(outU[:], ps_o[:], AF.Copy)
                            sums = nrm.tile([1, TQ], F32, tag="sums",
                                            name="sums")
                            nc.vector.tensor_copy(sums[:], ps_r[:])
                            recip = nrm.tile([1, TQ], F32, tag="recip",
                                             name="recip")
                            nc.vector.reciprocal_approx_fast(recip[:], sums[:])
                            bcast = nrm.tile([P, TQ], F32, tag="bcast",
                                             name="bcast")
                            nc.gpsimd.partition_broadcast(bcast[:], recip[:])
                            nc.vector.tensor_mul(outT[h][:, qsl], outU[:],
                                                 bcast[:])

                # =============== Phase F: projection ===============
                with tc.tile_pool(name="psPj", bufs=6, space="PSUM") as psPj, \
                     tc.tile_pool(name="ystg", bufs=3) as ystg:
                    for t in range(NT):
                        pss = [psPj.tile([P, TQ], F32, tag="psproj",
                                         name="psy") for _ in range(NQ)]
                        for h in range(NH):
                            for cn in range(NQ):
                                nc.tensor.matmul(
                                    pss[cn][:],
                                    lhsT=outT[h][:, t * P:(t + 1) * P],
                                    rhs=wp_b[:, h, cn * TQ:(cn + 1) * TQ],
                                    start=(h == 0), stop=(h == NH - 1))
                        ys = ystg.tile([P, T], F32, tag="ystage", name="ys")
                        for cn in range(NQ):
                            nc.scalar.activation(
                                ys[:, cn * TQ:(cn + 1) * TQ], pss[cn][:],
                                AF.Copy)
                        eng = nc.sync if t % 2 == 0 else nc.scalar
                        eng.dma_start(y[t * P:(t + 1) * P, :], ys[:])

    nc.compile()
    return nc


def _get_nc():
    global _CACHED_NC
    if _CACHED_NC is None:
        _CACHED_NC = build_nc()
    return _CACHED_NC


LAST_RESULTS = None


def kernel(x, cos, sin, W_attn, W_proj):
    global LAST_RESULTS
    x = np.asarray(x, np.float32)
    cos = np.asarray(cos, np.float32)
    sin = np.asarray(sin, np.float32)
    W_attn = np.asarray(W_attn, np.float32)
    W_proj = np.asarray(W_proj, np.float32)
    B = x.shape[0]

    cosT = np.ascontiguousarray(cos.T).astype(BF)          # [D, T]
    sinTf = np.ascontiguousarray(sin.T).copy()
    sinTf[: D // 2] *= -1.0                                # sign-folded rotate
    sinT = sinTf.astype(BF)

    xTs = [np.ascontiguousarray(x[b].T).astype(BF) for b in range(B)]
    in_maps = []
    for b in range(B):
        for g in range(4):
            csl = slice(g * 512, (g + 1) * 512)
            wqk2 = np.concatenate([W_attn[:, csl], W_attn[:, C:][:, csl]],
                                  axis=1).astype(BF)       # [C, 1024]
            # pack [8, 128, 16, 128]: wqkr[m, p, ko, j] = wqk2[128*ko+p, 128*m+j]
            wqkr = np.ascontiguousarray(
                wqk2.reshape(KO, P, 8, P).transpose(2, 1, 0, 3))
            wv2 = W_attn[:, 2 * C:][:, csl].astype(BF)     # [C, 512]
            wvr = np.ascontiguousarray(
                wv2.reshape(KO, P, NH * D).transpose(1, 0, 2))  # [128,16,512]
            wp2 = W_proj[g * 512:(g + 1) * 512, :].astype(BF)   # [512, C]
            wpr = np.ascontiguousarray(
                wp2.reshape(NH, P, C).transpose(1, 0, 2))       # [128,4,2048]
            in_maps.append({"xT": xTs[b], "wqk": wqkr, "wv": wvr, "wp": wpr,
                            "cosT": cosT, "sinT": sinT})

    nc = _get_nc()
    res = run_bass_kernel_spmd(nc, in_maps, core_ids=list(range(8)),
                               trace=TRACE)
    LAST_RESULTS = res

    out = np.zeros((B, T, C), np.float32)
    for b in range(B):
        acc = res.results[b * 4 + 0]["y"].astype(np.float32)
        for g in range(1, 4):
            acc = acc + res.results[b * 4 + g]["y"]
        out[b] = acc
    return out


# revision 14
# speedup vs baseline: 1.3821x; 1.0020x over previous
# Multi-head self-attention (B=2, T=2048, C=2048, H=16) on 8 trn2 NeuronCores.
# Sharding: core = (batch b, head-group g) with 4 heads per core.
# Inputs are pre-cast to bf16 and packed DMA-friendly on the host (the device
# would do the identical round-to-nearest cast before its bf16 matmuls).
# Per-core program (Tile framework, bf16 matmuls with fp32 PSUM accumulation):
#   qk^T = W_qk^T @ x^T   (lhsT = W chunks, rhs = x^T)      -> [D, T] per head
#   v    = x @ W_v        (lhsT = x^T chunks, rhs = W_v)    -> [T, D] natural
#   RoPE on q^T/k^T via half-swap DMA + elementwise mul/add (in place)
#   S^T tile = k_rope^T.T @ q_rope^T ; E^T = exp(scale*S^T) (causal)
#   out^T = v.T @ E^T ; rowsums via ones-matmul on DVE-paired E tiles
#   normalize off the PSUM path: evac unscaled, scale by bcast(1/sums) in SBUF
#   y_partial = out_heads^T.T @ W_p rows  -> [T, C], host sums 4 partials.
import sys

import numpy as np
import ml_dtypes

for _p in ("/opt/trn_rl_repo",):
    if _p not in sys.path:
        sys.path.append(_p)

import concourse.bass as bass
import concourse.mybir as mybir
import concourse.tile as tile
from concourse import bacc
from concourse.bass_utils import run_bass_kernel_spmd

P = 128
T = 2048
C = 2048
D = 128
NH = 4            # heads per core
KO = C // P       # 16 contraction chunks
TQ = 512          # q-tile width
NQ = T // TQ      # 4
NT = T // P       # 16 t-subtiles
SCALE = float(np.float32(1.0) / np.sqrt(np.float32(D)))

F32 = mybir.dt.float32
BF16 = mybir.dt.bfloat16
AF = mybir.ActivationFunctionType
BF = ml_dtypes.bfloat16

TRACE = False
_CACHED_NC = None


def _tri_mask_np():
    p = np.arange(P)[:, None]
    q = np.arange(P)[None, :]
    return (p <= q).astype(BF)


def build_nc():
    nc = bacc.Bacc("TRN2", target_bir_lowering=False, debug=False,
                   enable_asserts=False)

    # bf16 inputs, packed so every DMA moves >=4KB contiguous per partition
    xT_d = nc.dram_tensor("xT", [C, T], BF16, kind="ExternalInput")
    wqk_d = nc.dram_tensor("wqk", [8, P, KO, P], BF16, kind="ExternalInput")
    wv_d = nc.dram_tensor("wv", [P, KO, NH * D], BF16, kind="ExternalInput")
    wp_d = nc.dram_tensor("wp", [P, NH, C], BF16, kind="ExternalInput")
    cos_d = nc.dram_tensor("cosT", [D, T], BF16, kind="ExternalInput")
    sin_d = nc.dram_tensor("sinT", [D, T], BF16, kind="ExternalInput")
    y_d = nc.dram_tensor("y", [T, C], F32, kind="ExternalOutput")

    mask_d = nc.inline_tensor(_tri_mask_np(), name="trimask")

    xT = xT_d.ap().rearrange("(ko p) t -> p ko t", p=P)          # [128,16,2048]
    wqk = wqk_d.ap()
    y = y_d.ap()

    with tile.TileContext(nc) as tc:
        with (
            tc.tile_pool(name="glob", bufs=1) as glob,
            tc.tile_pool(name="rawp", bufs=1) as rawp,
        ):
            ones_sb = glob.tile([P, 1], BF16, tag="ones")
            nc.vector.memset(ones_sb[:], 1.0)
            v_b = [glob.tile([P, NT, P], BF16, tag=f"v_b{h}", name=f"v_b{h}")
                   for h in range(NH)]
            raw = [rawp.tile([P, T], BF16, tag=f"raw{m}", name=f"raw{m}")
                   for m in range(8)]

            # =============== Phase B: qkv matmuls + RoPE ===============
            with tc.tile_pool(name="loadB", bufs=1) as lB, \
                 tc.tile_pool(name="shufp", bufs=1) as shufp, \
                 tc.tile_pool(name="psB", bufs=6, space="PSUM") as psB:

                xb = lB.tile([P, KO, T], BF16, tag="xb")
                wqk_b = [lB.tile([P, KO, P], BF16, tag=f"wqk_b{m}",
                                 name=f"wqk_b{m}") for m in range(8)]
                # interleave weight/x loads; arrival order matches use order
                nc.sync.dma_start(wqk_b[0][:, 0:4, :], wqk[0, :, 0:4, :])
                nc.sync.dma_start(xb[:, 0, :], xT[:, 0, :])
                nc.sync.dma_start(wqk_b[0][:, 4:KO, :], wqk[0, :, 4:KO, :])
                for m in range(1, 8):
                    nc.sync.dma_start(wqk_b[m][:], wqk[m])
                    nc.sync.dma_start(xb[:, m, :], xT[:, m, :])
                for ko in range(8, KO):
                    nc.sync.dma_start(xb[:, ko, :], xT[:, ko, :])
                cos_b = lB.tile([P, T], BF16, tag="cos_b")
                sin_b = lB.tile([P, T], BF16, tag="sin_b")
                nc.sync.dma_start(cos_b[:], cos_d.ap())
                nc.sync.dma_start(sin_b[:], sin_d.ap())
                wv_b = lB.tile([P, KO, NH * D], BF16, tag="wv_b")
                nc.sync.dma_start(wv_b[:], wv_d.ap())

                # qk matmuls; k-order staggered by m so each group consumes
                # chunks roughly in DMA-arrival order
                for m in range(8):
                    pss = [psB.tile([P, TQ], F32, tag="psBig", name="psqk")
                           for _ in range(NQ)]
                    kos = [(2 * m + i) % KO for i in range(KO)]
                    for i, ko in enumerate(kos):
                        for n in range(NQ):
                            nc.tensor.matmul(
                                pss[n][:], lhsT=wqk_b[m][:, ko, :],
                                rhs=xb[:, ko, n * TQ:(n + 1) * TQ],
                                start=(i == 0), stop=(i == KO - 1))
                    for n in range(NQ):
                        nc.scalar.activation(
                            raw[m][:, n * TQ:(n + 1) * TQ], pss[n][:], AF.Copy)
                    # rope in place
                    r = raw[m]
                    sh = shufp.tile([P, T], BF16, tag="shuf", name="sh")
                    nc.sync.dma_start(sh[0:64, :], r[64:128, :])
                    nc.sync.dma_start(sh[64:128, :], r[0:64, :])
                    nc.vector.tensor_mul(sh[:], sh[:], sin_b[:])
                    nc.vector.tensor_mul(r[:], r[:], cos_b[:])
                    nc.vector.tensor_add(r[:], r[:], sh[:])

                # v matmuls (natural layout)
                for t in range(NT):
                    psv = psB.tile([P, TQ], F32, tag="psBig", name="psv")
                    for ko in range(KO):
                        nc.tensor.matmul(
                            psv[:], lhsT=xb[:, ko, t * P:(t + 1) * P],
                            rhs=wv_b[:, ko, :],
                            start=(ko == 0), stop=(ko == KO - 1))
                    for h in range(NH):
                        nc.scalar.activation(
                            v_b[h][:, t, :], psv[:, h * P:(h + 1) * P], AF.Copy)

            # =============== Phase D: attention ===============
            with tc.tile_pool(name="attnp", bufs=1) as ap_, \
                 tc.tile_pool(name="etp", bufs=3) as etp, \
                 tc.tile_pool(name="nrm", bufs=2) as nrm:

                mask_sb = ap_.tile([P, P], BF16, tag="trimask")
                nc.sync.dma_start(mask_sb[:], mask_d.ap())
                outT = [ap_.tile([P, T], BF16, tag=f"outT{h}", name=f"outT{h}")
                        for h in range(NH)]
                wp_b = ap_.tile([P, NH, C], BF16, tag="wp_b")
                nc.sync.dma_start(wp_b[:], wp_d.ap())

                with tc.tile_pool(name="psS2", bufs=2, space="PSUM") as psS2, \
                     tc.tile_pool(name="psO", bufs=3, space="PSUM") as psO, \
                     tc.tile_pool(name="psR", bufs=1, space="PSUM") as psR:
                    for h in range(NH):
                        qr = raw[h]
                        kr = raw[4 + h]
                        # qo descending: dense large-qo tiles first, so the
                        # latency-bound qo=0 chain overlaps other work
                        for qo in reversed(range(NQ)):
                            qsl = slice(qo * TQ, (qo + 1) * TQ)
                            nfull = 4 * qo
                            ps_o = psO.tile([P, TQ], F32, tag="psout",
                                            name="ps_o")
                            ps_r = psR.tile([1, TQ], F32, tag="psrow",
                                            name="ps_r")
                            # diagonal chunks first: their exp/mask latency
                            # chains hide under the dense pairs that follow
                            for jr in range(4):
                                j = nfull + jr
                                off = jr * P
                                ps2 = psS2.tile([P, 2, TQ], F32, tag="psscore",
                                                name="ps2d")
                                nc.tensor.matmul(
                                    ps2[:, 0, off:TQ],
                                    lhsT=kr[:, j * P:(j + 1) * P],
                                    rhs=qr[:, qo * TQ + off:(qo + 1) * TQ],
                                    start=True, stop=True)
                                et = etp.tile([P, TQ], BF16, tag="et1",
                                              name="et1", bufs=6)
                                nc.scalar.activation(et[:, off:TQ],
                                                     ps2[:, 0, off:TQ], AF.Exp,
                                                     scale=SCALE)
                                nc.vector.tensor_mul(et[:, off:off + P],
                                                     et[:, off:off + P],
                                                     mask_sb[:])
                                last = (jr == 3) and nfull == 0
                                nc.tensor.matmul(
                                    ps_o[:, off:TQ], lhsT=v_b[h][:, j, :],
                                    rhs=et[:, off:TQ],
                                    start=(jr == 0), stop=last)
                                nc.tensor.matmul(
                                    ps_r[:, off:TQ], lhsT=ones_sb[:, 0:1],
                                    rhs=et[:, off:TQ],
                                    start=(jr == 0), stop=last)
                            for pr in range(nfull // 2):
                                ps2 = psS2.tile([P, 2, TQ], F32, tag="psscore",
                                                name="ps2")
                                for s in range(2):
                                    j = 2 * pr + s
                                    nc.tensor.matmul(
                                        ps2[:, s, :],
                                        lhsT=kr[:, j * P:(j + 1) * P],
                                        rhs=qr[:, qsl], start=True, stop=True)
                                et2 = etp.tile([P, 2, TQ], BF16, tag="et2",
                                               name="et2", bufs=5)
                                nc.scalar.activation(et2[:], ps2[:], AF.Exp,
                                                     scale=SCALE)
                                last = (pr == nfull // 2 - 1)
                                for s in range(2):
                                    j = 2 * pr + s
                                    nc.tensor.matmul(
                                        ps_o[:], lhsT=v_b[h][:, j, :],
                                        rhs=et2[:, s, :],
                                        start=False, stop=(last and s == 1))
                                esum = etp.tile([P, TQ], BF16, tag="esum",
                                                name="esum", bufs=3)
                                nc.vector.tensor_add(esum[:], et2[:, 0, :],
                                                     et2[:, 1, :])
                                nc.tensor.matmul(
                                    ps_r[:], lhsT=ones_sb[:, 0:1],
                                    rhs=esum[:],
                                    start=False, stop=last)
                            # evacuate unscaled output now (frees the PSUM
                            # bank fast); normalize later in SBUF, off the
                            # PE critical path
                            outU = nrm.tile([P, TQ], F32, tag="outU",
                                            name="outU", bufs=3)
                            nc.scalar.activation(outU[:], ps_o[:], AF.Copy)
                            sums = nrm.tile([1, TQ], F32, tag="sums",
                                            name="sums")
                            nc.vector.tensor_copy(sums[:], ps_r[:])
                            recip = nrm.tile([1, TQ], F32, tag="recip",
                                             name="recip")
                            nc.vector.reciprocal_approx_fast(recip[:], sums[:])
                            bcast = nrm.tile([P, TQ], F32, tag="bcast",
                                             name="bcast")
                            nc.gpsimd.partition_broadcast(bcast[:], recip[:])
                            nc.vector.tensor_mul(outT[h][:, qsl], outU[:],
                                                 bcast[:])

                # =============== Phase F: projection ===============
                with tc.tile_pool(name="psPj", bufs=6, space="PSUM") as psPj, \
                     tc.tile_pool(name="ystg", bufs=3) as ystg:
                    for t in range(NT):
                        pss = [psPj.tile([P, TQ], F32, tag="psproj",
                                         name="psy") for _ in range(NQ)]
                        for h in range(NH):
                            for cn in range(NQ):
                                nc.tensor.matmul(
                                    pss[cn][:],
                                    lhsT=outT[h][:, t * P:(t + 1) * P],
                                    rhs=wp_b[:, h, cn * TQ:(cn + 1) * TQ],
                                    start=(h == 0), stop=(h == NH - 1))
                        ys = ystg.tile([P, T], F32, tag="ystage", name="ys")
                        for cn in range(NQ):
                            nc.scalar.activation(
                                ys[:, cn * TQ:(cn + 1) * TQ], pss[cn][:],
                                AF.Copy)
                        eng = nc.sync if t % 2 == 0 else nc.scalar
                        eng.dma_start(y[t * P:(t + 1) * P, :], ys[:])

    nc.compile()
    return nc


def _get_nc():
    global _CACHED_NC
    if _CACHED_NC is None:
        _CACHED_NC = build_nc()
    return _CACHED_NC


LAST_RESULTS = None


def kernel(x, cos, sin, W_attn, W_proj):
    global LAST_RESULTS
    x = np.asarray(x, np.float32)
    cos = np.asarray(cos, np.float32)
    sin = np.asarray(sin, np.float32)
    W_attn = np.asarray(W_attn, np.float32)
    W_proj = np.asarray(W_proj, np.float32)
    B = x.shape[0]

    cosT = np.ascontiguousarray(cos.T).astype(BF)          # [D, T]
    sinTf = np.ascontiguousarray(sin.T).copy()
    sinTf[: D // 2] *= -1.0                                # sign-folded rotate
    sinT = sinTf.astype(BF)

    xTs = [np.ascontiguousarray(x[b].T).astype(BF) for b in range(B)]
    in_maps = []
    for b in range(B):
        for g in range(4):
            csl = slice(g * 512, (g + 1) * 512)
            wqk2 = np.concatenate([W_attn[:, csl], W_attn[:, C:][:, csl]],
                                  axis=1).astype(BF)       # [C, 1024]
            # pack [8, 128, 16, 128]: wqkr[m, p, ko, j] = wqk2[128*ko+p, 128*m+j]
            wqkr = np.ascontiguousarray(
                wqk2.reshape(KO, P, 8, P).transpose(2, 1, 0, 3))
            wv2 = W_attn[:, 2 * C:][:, csl].astype(BF)     # [C, 512]
            wvr = np.ascontiguousarray(
                wv2.reshape(KO, P, NH * D).transpose(1, 0, 2))  # [128,16,512]
            wp2 = W_proj[g * 512:(g + 1) * 512, :].astype(BF)   # [512, C]
            wpr = np.ascontiguousarray(
                wp2.reshape(NH, P, C).transpose(1, 0, 2))       # [128,4,2048]
            in_maps.append({"xT": xTs[b], "wqk": wqkr, "wv": wvr, "wp": wpr,
                            "cosT": cosT, "sinT": sinT})

    nc = _get_nc()
    res = run_bass_kernel_spmd(nc, in_maps, core_ids=list(range(8)),
                               trace=TRACE)
    LAST_RESULTS = res

    out = np.zeros((B, T, C), np.float32)
    for b in range(B):
        acc = res.results[b * 4 + 0]["y"].astype(np.float32)
        for g in range(1, 4):
            acc = acc + res.results[b * 4 + g]["y"]
        out[b] = acc
    return out
